# revision 4
# baseline (speedup 1.0000x reference)
"""CLIP attention (B=2, S=2048, H=768, 12 heads) on 8 trn2 NeuronCores.

Sharding: data-parallel over batch (2) x tensor-parallel over head groups
(4 groups of 3 heads).  Each core computes, for its (batch, head-group):
    q = x @ Wq_g * 1/sqrt(64) (+ bq_g scaled)      [2048, 192]
    k = x @ Wk_g                                    [2048, 192]
    v = x @ Wv_g                                    [2048, 192]
    per head: P' = exp(q k^T)   (no max subtraction; logits are O(1))
              O' = P'^T-weighted sums via ones-augmented V  -> O'^T, s
              O^T = O'^T / s
    y_partial = O @ Wo_g                            [2048, 768]
Host sums the 4 head-group partials per batch and adds the exactly-folded
bias terms (bk drops out of softmax; bv/bo fold to a constant row).

Matmul convention: nc.tensor.matmul(out, lhsT, rhs) => out = lhsT.T @ rhs,
contraction over the partition dim of both operands.
"""

import sys

if "/opt/trn_rl_repo" not in sys.path:
    sys.path.insert(0, "/opt/trn_rl_repo")

import numpy as np
import ml_dtypes

import concourse.bacc as bacc
import concourse.tile as tile
from concourse import mybir
from concourse.bass_utils import run_bass_kernel_spmd

BF16 = mybir.dt.bfloat16
F32 = mybir.dt.float32

S = 2048          # sequence length
C = 768           # hidden
NH = 12           # total heads
HD = 64           # head dim
NCORES = 8
GROUPS = 4        # head groups (tensor parallel)
HPG = NH // GROUPS          # heads per group = 3
GF = HPG * HD               # group feature width = 192
NCC = C // 128              # contraction chunks = 6
NQB = S // 128              # token blocks = 16
NKB = S // 128              # key blocks = 16


def build_program():
    nc = bacc.Bacc("TRN2", target_bir_lowering=False, debug=False)

    x = nc.dram_tensor("x", (S, C), BF16, kind="ExternalInput").ap()
    wq = nc.dram_tensor("wq", (C, GF), BF16, kind="ExternalInput").ap()
    wk = nc.dram_tensor("wk", (C, GF), BF16, kind="ExternalInput").ap()
    wv = nc.dram_tensor("wv", (C, GF), BF16, kind="ExternalInput").ap()
    wo = nc.dram_tensor("wo", (GF, C), BF16, kind="ExternalInput").ap()
    bq = nc.dram_tensor("bq", (1, GF), BF16, kind="ExternalInput").ap()
    out = nc.dram_tensor("out", (S, C), F32, kind="ExternalOutput").ap()

    with tile.TileContext(nc) as tc:
        with tc.tile_pool(name="consts", bufs=1) as consts:
            # x^T in SBUF: [p, cchunk, token], p+128*cchunk = feature
            xT = consts.tile([128, NCC, S], BF16)
            for c in range(NCC):
                nc.sync.dma_start_transpose(
                    out=xT[:, c, :], in_=x[:, c * 128 : (c + 1) * 128]
                )
            wq_sb = consts.tile([128, NCC, GF], BF16)
            wk_sb = consts.tile([128, NCC, GF], BF16)
            wv_sb = consts.tile([128, NCC, GF], BF16)
            for w_sb, w_dram in ((wq_sb, wq), (wk_sb, wk), (wv_sb, wv)):
                for c in range(NCC):
                    nc.sync.dma_start(
                        out=w_sb[:, c, :], in_=w_dram[c * 128 : (c + 1) * 128, :]
                    )
            # Wo as [d, head, n]
            wo_sb = consts.tile([HD, HPG, C], BF16)
            for h in range(HPG):
                nc.sync.dma_start(
                    out=wo_sb[:, h, :], in_=wo[h * HD : (h + 1) * HD, :]
                )
            bq_sb = consts.tile([1, GF], BF16)
            nc.sync.dma_start(out=bq_sb[:], in_=bq[:])

            ones_bf = consts.tile([1, 512], BF16)
            nc.vector.memset(ones_bf[:], 1.0)
            ones64 = consts.tile([1, HD], F32)
            nc.vector.memset(ones64[:], 1.0)

            # per-head transposed projections [d, token]
            qT = [consts.tile([HD, S], BF16, name=f"qT{h}") for h in range(HPG)]
            kT = [consts.tile([HD, S], BF16, name=f"kT{h}") for h in range(HPG)]
            # V with ones column: [token_p, tokblock, head, 65]
            vS = consts.tile([128, NKB, HPG, HD + 1], BF16)
            # normalized O^T per head [d, token]
            oT = [consts.tile([HD, S], BF16, name=f"oT{h}") for h in range(HPG)]

            # ---------------- QKV projections ----------------
            with tc.tile_pool(name="proj_ps", bufs=3, space="PSUM") as pp, \
                 tc.tile_pool(name="vproj_ps", bufs=2, space="PSUM") as vpp:
                # Q^T and K^T: M-chunks of 96 over the 192 group features
                def qk_copy(dst_list, piece, m, n):
                    # piece rows = global features m*96..m*96+95.
                    # PSUM reads must start at a 32-aligned partition with a
                    # span that doesn't cross the 32-block alignment rule.
                    n0 = n * 512
                    if m == 0:
                        nc.vector.tensor_copy(
                            dst_list[0][:, n0 : n0 + 512], piece[0:64, :]
                        )
                        nc.vector.tensor_copy(
                            dst_list[1][0:32, n0 : n0 + 512], piece[64:96, :]
                        )
                    else:
                        nc.vector.tensor_copy(
                            dst_list[1][32:64, n0 : n0 + 512], piece[0:32, :]
                        )
                        nc.vector.tensor_copy(
                            dst_list[2][0:32, n0 : n0 + 512], piece[32:64, :]
                        )
                        nc.vector.tensor_copy(
                            dst_list[2][32:64, n0 : n0 + 512], piece[64:96, :]
                        )

                for m in range(2):
                    for proj, (w_sb, dst, with_bias) in enumerate(
                        ((wq_sb, qT, True), (wk_sb, kT, False))
                    ):
                        for n in range(S // 512):
                            ps = pp.tile([96, 512], F32, tag="ps")
                            for c in range(NCC):
                                nc.tensor.matmul(
                                    ps[:],
                                    w_sb[:, c, m * 96 : (m + 1) * 96],
                                    xT[:, c, n * 512 : (n + 1) * 512],
                                    start=(c == 0),
                                    stop=(c == NCC - 1 and not with_bias),
                                )
                            if with_bias:
                                nc.tensor.matmul(
                                    ps[:],
                                    bq_sb[:, m * 96 : (m + 1) * 96],
                                    ones_bf[:],
                                    start=False,
                                    stop=True,
                                )
                            qk_copy(dst, ps, m, n)

                # V projection: per token block
                for t in range(NQB):
                    vps = vpp.tile([128, GF], F32, tag="vps")
                    for c in range(NCC):
                        nc.tensor.matmul(
                            vps[:],
                            xT[:, c, t * 128 : (t + 1) * 128],
                            wv_sb[:, c, :],
                            start=(c == 0),
                            stop=(c == NCC - 1),
                        )
                    nc.vector.tensor_copy(
                        vS[:, t, :, 0:HD],
                        vps[:].rearrange("p (h d) -> p h d", h=HPG),
                    )
                    nc.vector.memset(vS[:, t, :, HD : HD + 1], 1.0)

            # ---------------- attention ----------------
            with tc.tile_pool(name="lt_ps", bufs=2, space="PSUM") as ltp, \
                 tc.tile_pool(name="o_ps", bufs=1, space="PSUM") as opp, \
                 tc.tile_pool(name="att_sb", bufs=3) as asb:
                for h in range(HPG):
                    o_ps = opp.tile([HD + 1, S], F32, tag="o")
                    for kb in range(NKB):
                        for qh in range(2):
                            q0 = qh * 1024
                            lt = ltp.tile([128, 1024], F32, tag="lt")
                            for nn in range(2):
                                nc.tensor.matmul(
                                    lt[:, nn * 512 : (nn + 1) * 512],
                                    kT[h][:, kb * 128 : (kb + 1) * 128],
                                    qT[h][:, q0 + nn * 512 : q0 + (nn + 1) * 512],
                                    start=True,
                                    stop=True,
                                )
                            elt = asb.tile([128, 1024], BF16, tag="elt")
                            nc.scalar.activation(
                                elt[:], lt[:], mybir.ActivationFunctionType.Exp
                            )
                            for nn in range(2):
                                nc.tensor.matmul(
                                    o_ps[:, q0 + nn * 512 : q0 + (nn + 1) * 512],
                                    vS[:, kb, h, :],
                                    elt[:, nn * 512 : (nn + 1) * 512],
                                    start=(kb == 0),
                                    stop=(kb == NKB - 1),
                                )
                    # normalize: oT = O'^T * (1/s) broadcast across partitions
                    recip = asb.tile([1, S], F32, tag="recip")
                    nc.vector.reciprocal(recip[:], o_ps[HD : HD + 1, :])
                    for qh in range(2):
                        q0 = qh * 1024
                        bc = ltp.tile([HD, 1024], F32, tag="lt")
                        for nn in range(2):
                            nc.tensor.matmul(
                                bc[:, nn * 512 : (nn + 1) * 512],
                                ones64[:],
                                recip[:, q0 + nn * 512 : q0 + (nn + 1) * 512],
                                start=True,
                                stop=True,
                            )
                        # DVE may read only one PSUM operand per instruction
                        bc_sb = asb.tile([HD, 1024], F32, tag="bc")
                        nc.vector.tensor_copy(bc_sb[:], bc[:])
                        nc.vector.tensor_mul(
                            oT[h][:, q0 : q0 + 1024],
                            o_ps[0:HD, q0 : q0 + 1024],
                            bc_sb[:],
                        )

            # ---------------- output projection ----------------
            with tc.tile_pool(name="y_ps", bufs=3, space="PSUM") as yp, \
                 tc.tile_pool(name="y_sb", bufs=3) as ysb:
                for t in range(NQB):
                    yps = yp.tile([128, C], F32, tag="y")
                    for n0, nw in ((0, 512), (512, 256)):
                        for h in range(HPG):
                            nc.tensor.matmul(
                                yps[:, n0 : n0 + nw],
                                oT[h][:, t * 128 : (t + 1) * 128],
                                wo_sb[:, h, n0 : n0 + nw],
                                start=(h == 0),
                                stop=(h == HPG - 1),
                            )
                    ys = ysb.tile([128, C], F32, tag="ys")
                    nc.vector.tensor_copy(ys[:], yps[:])
                    nc.sync.dma_start(
                        out=out[t * 128 : (t + 1) * 128, :], in_=ys[:]
                    )

    nc.compile()
    return nc


_COMPILED_NC = None


def _get_nc():
    global _COMPILED_NC
    if _COMPILED_NC is None:
        _COMPILED_NC = build_program()
    return _COMPILED_NC


def make_in_maps(x, Wq, bq, Wk, bk, Wv, bv, Wo, bo):
    scale = 1.0 / np.sqrt(HD)
    bf = ml_dtypes.bfloat16
    x_bf = [np.ascontiguousarray(x[b]).astype(bf) for b in range(x.shape[0])]
    in_maps = []
    for c in range(NCORES):
        b, g = divmod(c, GROUPS)
        cols = slice(g * GF, (g + 1) * GF)
        in_maps.append(
            {
                "x": x_bf[b],
                "wq": np.ascontiguousarray(Wq[:, cols] * scale).astype(bf),
                "wk": np.ascontiguousarray(Wk[:, cols]).astype(bf),
                "wv": np.ascontiguousarray(Wv[:, cols]).astype(bf),
                "wo": np.ascontiguousarray(Wo[cols, :]).astype(bf),
                "bq": np.ascontiguousarray(bq[cols] * scale).reshape(1, GF).astype(bf),
            }
        )
    return in_maps


def gather_output(results, x, Wv, bv, Wo, bo):
    B = x.shape[0]
    out = np.zeros((B, S, C), dtype=np.float32)
    for c in range(NCORES):
        b, _ = divmod(c, GROUPS)
        out[b] += results[c]["out"]
    # exact bias folds: bk cancels in softmax; v-bias -> bv @ Wo; + bo
    out += (np.asarray(bv, np.float32) @ np.asarray(Wo, np.float32)
            + np.asarray(bo, np.float32))
    return out


def kernel(x, Wq, bq, Wk, bk, Wv, bv, Wo, bo):
    x = np.asarray(x)
    nc = _get_nc()
    in_maps = make_in_maps(x, Wq, bq, Wk, bk, Wv, bv, Wo, bo)
    res = run_bass_kernel_spmd(nc, in_maps, core_ids=list(range(NCORES)))
    return gather_output(res.results, x, Wv, bv, Wo, bo)
